# revision 1
# baseline (speedup 1.0000x reference)
"""Trainium2 Bass kernel for nn_Decoder (ragged LSTM decoder), 8-core SPMD.

Strategy: data-parallel over batch (16 batch rows per core). Per core:
  Phase A (parallel over t): ragged word-avg via banded matmul against
    on-device-built coefficient matrices; pos-embedding gather via one-hot
    matmul; z = tanh(x @ combine_W.T + b); G_in = z @ W_ih.T + biases -> DRAM.
    Also produces enc.T tiles (via identity matmul) -> DRAM for phase C.
  Phase B (sequential scan, T=512): g = G_in[t] + h @ W_hh.T (36 128x128
    matmuls/step, gates packed [128 part, 12*16 cols]); LSTM cell on ACT/DVE;
    h2 history -> DRAM.
  Phase C (parallel): logits = [h2, e_t] @ out_W.T in [cols, label] layout,
    t==0 appID fixup, log_softmax along free axis, DMA out.
Column order everywhere is t-major: col = t*16 + b_local.
"""
import sys
sys.path.insert(0, "/opt/trn_rl_repo")

import numpy as np

B, T, H = 128, 512, 384
D_ENC, HID = 768, 768
POS_SIZE, POS_DIM, LABEL = 64, 128, 128
APP_ID = 3
NCORES = 8
BC = B // NCORES          # 16 batch rows per core
COLS = T * BC             # 8192 (t-major)
NTT = T // 128            # 4 t-tiles
F32 = None                # set after mybir import

_COMPILED = None


def _build(reps=1, phases='abc'):
    import concourse.bass as bass
    import concourse.mybir as mybir
    import concourse.tile as tile
    from concourse import bacc
    from contextlib import ExitStack

    f32 = mybir.dt.float32
    AF = mybir.ActivationFunctionType
    ALU = mybir.AluOpType

    nc = bacc.Bacc(None, target_bir_lowering=False, debug=False,
                   num_devices=NCORES)

    def param(name, shape):
        return nc.declare_dram_parameter(name, list(shape), f32, isOutput=False)

    enc = param("enc", [BC, T, D_ENC])
    sreld = param("sreld", [BC, T])
    srelc = param("srelc", [BC, T])
    recipv = param("recipv", [BC, T])
    pidcol = param("pidcol", [COLS])
    combWT = param("combWT", [7, 128, HID])        # combine_W.T split on K
    wihT = param("wihT", [6, 128, 4 * H])
    whhT = param("whhT", [3, 128, 4 * H])
    outWhT = param("outWhT", [3, 128, LABEL])
    outWeT = param("outWeT", [6, 128, LABEL])
    posw = param("posw", [POS_SIZE, POS_DIM])
    combb = param("combb", [6, 128])
    biassum = param("biassum", [12, 128])
    mlt = param("mlt", [128, 128])                  # [p,t] = 1.0 if p<t
    iota = param("iota", [128])
    id128 = param("id128", [128, 128])

    out = nc.declare_dram_parameter("out", [BC, T, LABEL], f32, isOutput=True)

    encT_d = nc.dram_tensor("encT_d", [6, 128, T, BC], f32)
    gin_d = nc.dram_tensor("gin_d", [128, T, 12, BC], f32)
    h2_d = nc.dram_tensor("h2_d", [128, 3, T * BC], f32)

    with tile.TileContext(nc) as tc, ExitStack() as top:
        singles = top.enter_context(tc.tile_pool(name="singles", bufs=1))

        # ---- resident weights/constants in SBUF ----
        combWT_sb = singles.tile([128, 7, HID], f32)
        wihT_sb = singles.tile([128, 6, 4 * H], f32)
        whhT_sb = singles.tile([128, 3, 4 * H], f32)
        outWhT_sb = singles.tile([128, 3, LABEL], f32)
        outWeT_sb = singles.tile([128, 6, LABEL], f32)
        posw_sb = singles.tile([POS_SIZE, POS_DIM], f32)
        combb_sb = singles.tile([128, 6], f32)
        biassum_sb = singles.tile([128, 12], f32)
        mlt_sb = singles.tile([128, 128], f32)
        iota_sb = singles.tile([128, 1], f32)
        id_sb = singles.tile([128, 128], f32)
        nc.sync.dma_start(out=combWT_sb, in_=combWT.ap().rearrange("k p m -> p k m"))
        nc.sync.dma_start(out=wihT_sb, in_=wihT.ap().rearrange("k p m -> p k m"))
        nc.sync.dma_start(out=whhT_sb, in_=whhT.ap().rearrange("k p m -> p k m"))
        nc.sync.dma_start(out=outWhT_sb, in_=outWhT.ap().rearrange("k p m -> p k m"))
        nc.sync.dma_start(out=outWeT_sb, in_=outWeT.ap().rearrange("k p m -> p k m"))
        nc.sync.dma_start(out=posw_sb, in_=posw.ap())
        nc.sync.dma_start(out=combb_sb, in_=combb.ap().rearrange("m p -> p m"))
        nc.sync.dma_start(out=biassum_sb, in_=biassum.ap().rearrange("m p -> p m"))
        nc.sync.dma_start(out=mlt_sb, in_=mlt.ap())
        nc.sync.dma_start(out=iota_sb, in_=iota.ap().rearrange("(p o) -> p o", o=1))
        nc.sync.dma_start(out=id_sb, in_=id128.ap())

        for _rep in range(reps):
            # ================= Phase A =================
            with ExitStack() as pa:
              if 'a' in phases:
                xt_pool = pa.enter_context(tc.tile_pool(name="xt", bufs=1))
                a_pool = pa.enter_context(tc.tile_pool(name="awork", bufs=2))
                bc_pool = pa.enter_context(tc.tile_pool(name="abcast", bufs=3))
                z_pool = pa.enter_context(tc.tile_pool(name="zt", bufs=2))
                g_pool = pa.enter_context(tc.tile_pool(name="ginw", bufs=3))
                ps1 = pa.enter_context(tc.tile_pool(name="psA1", bufs=2, space="PSUM"))
                psp = pa.enter_context(tc.tile_pool(name="psPos", bufs=1, space="PSUM"))
                ps3 = pa.enter_context(tc.tile_pool(name="psA3", bufs=2, space="PSUM"))
                ps4 = pa.enter_context(tc.tile_pool(name="psA4", bufs=2, space="PSUM"))

                for tt in range(NTT):
                    t0 = tt * 128
                    xT = xt_pool.tile([128, 7, 128, BC], f32)  # [p, ktile, t, b]

                    # --- A2: pos embedding via one-hot matmul ---
                    for c4 in range(4):
                        colb = t0 * BC + c4 * 512
                        pid_b = a_pool.tile([POS_SIZE, 512], f32, tag="pidb")
                        src = bass.AP(tensor=pidcol.ap().tensor,
                                      offset=pidcol.ap().offset + colb,
                                      ap=[[0, POS_SIZE], [1, 512]])
                        nc.gpsimd.dma_start(out=pid_b, in_=src)
                        oh = a_pool.tile([POS_SIZE, 512], f32, tag="oh")
                        nc.vector.tensor_scalar(oh, pid_b, iota_sb[0:POS_SIZE, :],
                                                None, ALU.is_equal)
                        pp = psp.tile([128, 512], f32, tag="pspos")
                        nc.tensor.matmul(pp, posw_sb, oh, start=True, stop=True)
                        nc.scalar.activation(
                            xT[:, 0, c4 * 32:(c4 + 1) * 32, :], pp, AF.Copy)

                    # --- A1: ragged word-average + enc transpose ---
                    for b in range(BC):
                        enc_sb = a_pool.tile([128, D_ENC], f32, tag="encin")
                        nc.sync.dma_start(out=enc_sb, in_=enc.ap()[b, t0:t0 + 128, :])
                        if tt > 0:
                            encpre = a_pool.tile([8, D_ENC], f32, tag="encpre")
                            nc.sync.dma_start(out=encpre,
                                              in_=enc.ap()[b, t0 - 8:t0, :])
                        sreld_b = bc_pool.tile([128, 128], f32, tag="sreldb")
                        recip_b = bc_pool.tile([128, 128], f32, tag="recipb")
                        nc.gpsimd.dma_start(
                            out=sreld_b,
                            in_=bass.AP(tensor=sreld.ap().tensor,
                                        offset=sreld.ap().offset + b * T + t0,
                                        ap=[[0, 128], [1, 128]]))
                        nc.gpsimd.dma_start(
                            out=recip_b,
                            in_=bass.AP(tensor=recipv.ap().tensor,
                                        offset=recipv.ap().offset + b * T + t0,
                                        ap=[[0, 128], [1, 128]]))
                        C = bc_pool.tile([128, 128], f32, tag="cmat")
                        nc.vector.scalar_tensor_tensor(
                            C, sreld_b, iota_sb, mlt_sb, ALU.is_le, ALU.mult)
                        nc.vector.tensor_mul(C, C, recip_b)
                        if tt > 0:
                            srelc_b = bc_pool.tile([8, 128], f32, tag="srelcb")
                            nc.gpsimd.dma_start(
                                out=srelc_b,
                                in_=bass.AP(tensor=srelc.ap().tensor,
                                            offset=srelc.ap().offset + b * T + t0,
                                            ap=[[0, 8], [1, 128]]))
                            Ccr = bc_pool.tile([8, 128], f32, tag="ccr")
                            nc.vector.scalar_tensor_tensor(
                                Ccr, srelc_b, iota_sb[0:8, :], recip_b[0:8, :],
                                ALU.is_le, ALU.mult)
                        for fc in range(6):
                            ps = ps1.tile([128, 256], f32, tag="psw")
                            lhs = enc_sb[:, fc * 128:(fc + 1) * 128]
                            nc.tensor.matmul(ps[:, 0:128], lhs, C,
                                             start=True, stop=(tt == 0))
                            if tt > 0:
                                nc.tensor.matmul(
                                    ps[:, 0:128],
                                    encpre[:, fc * 128:(fc + 1) * 128], Ccr,
                                    start=False, stop=True)
                            nc.tensor.matmul(ps[:, 128:256], lhs, id_sb,
                                             start=True, stop=True)
                            nc.vector.tensor_copy(xT[:, 1 + fc, :, b], ps[:, 0:128])
                            ecp = a_pool.tile([128, 128], f32, tag="ecp")
                            nc.scalar.activation(ecp, ps[:, 128:256], AF.Copy)
                            nc.sync.dma_start(out=encT_d.ap()[fc, :, t0:t0 + 128, b],
                                              in_=ecp)

                    # --- A3 + A4 per 512-col chunk ---
                    for c4 in range(4):
                        tg = t0 + c4 * 32
                        zT = z_pool.tile([128, 6, 512], f32)
                        for m in range(6):
                            ps = ps3.tile([128, 512], f32)
                            for k in range(7):
                                nc.tensor.matmul(
                                    ps, combWT_sb[:, k, m * 128:(m + 1) * 128],
                                    xT[:, k, c4 * 32:(c4 + 1) * 32, :],
                                    start=(k == 0), stop=(k == 6))
                            nc.scalar.activation(zT[:, m, :], ps, AF.Tanh,
                                                 bias=combb_sb[:, m:m + 1])
                        if tt == 0 and c4 == 0:
                            for m in range(6):
                                nc.vector.memset(zT[:, m, 0:BC], 0.0)
                        for j in range(12):
                            ps = ps4.tile([128, 512], f32)
                            for k in range(6):
                                nc.tensor.matmul(
                                    ps, wihT_sb[:, k, j * 128:(j + 1) * 128],
                                    zT[:, k, :], start=(k == 0), stop=(k == 5))
                            gsb = g_pool.tile([128, 32, BC], f32)
                            nc.scalar.activation(gsb, ps, AF.Identity,
                                                 bias=biassum_sb[:, j:j + 1])
                            nc.sync.dma_start(out=gin_d.ap()[:, tg:tg + 32, j, :],
                                              in_=gsb)

            # ================= Phase B: scan =================
            with ExitStack() as pb:
              if 'b' in phases:
                  ginr = pb.enter_context(tc.tile_pool(name="ginr", bufs=3))
                  hpool = pb.enter_context(tc.tile_pool(name="hp", bufs=3))
                  cpool = pb.enter_context(tc.tile_pool(name="cp", bufs=3))
                  gapool = pb.enter_context(tc.tile_pool(name="ga", bufs=3))
                  tmp = pb.enter_context(tc.tile_pool(name="stmp", bufs=6))
                  pss = pb.enter_context(tc.tile_pool(name="psS", bufs=2, space="PSUM"))

                  hT = hpool.tile([128, 3 * BC], f32, tag="h")
                  cT = cpool.tile([128, BC * 3], f32, tag="c")
                  nc.vector.memset(hT, 0.0)
                  nc.vector.memset(cT, 0.0)
                  SFUNC = [AF.Sigmoid, AF.Sigmoid, AF.Tanh, AF.Sigmoid]
                  for blk in range(T // 16):
                      gch = ginr.tile([128, 16, 12, BC], f32)
                      nc.sync.dma_start(out=gch,
                                        in_=gin_d.ap()[:, blk * 16:(blk + 1) * 16, :, :])
                      for s in range(16):
                          t = blk * 16 + s
                          psg = [pss.tile([128, 3 * BC], f32, tag=f"psg{gi}",
                                          name=f"psg{gi}")
                                 for gi in range(4)]
                          for gi in range(4):
                              for jj in range(3):
                                  j = gi * 3 + jj
                                  for k in range(3):
                                      nc.tensor.matmul(
                                          psg[gi][:, jj * BC:(jj + 1) * BC],
                                          whhT_sb[:, k, j * 128:(j + 1) * 128],
                                          hT[:, k * BC:(k + 1) * BC],
                                          start=(k == 0), stop=(k == 2))
                          gact = gapool.tile([128, 12 * BC], f32)
                          for gi in range(4):
                              gs = tmp.tile([128, 3 * BC], f32, tag="gs")
                              nc.vector.tensor_add(
                                  gs, psg[gi], gch[:, s, gi * 3:(gi + 1) * 3, :])
                              nc.scalar.activation(
                                  gact[:, gi * 3 * BC:(gi + 1) * 3 * BC], gs,
                                  SFUNC[gi])
                          i_a = gact[:, 0:3 * BC]
                          f_a = gact[:, 3 * BC:6 * BC]
                          g_a = gact[:, 6 * BC:9 * BC]
                          o_a = gact[:, 9 * BC:12 * BC]
                          t1 = tmp.tile([128, 3 * BC], f32, tag="t1")
                          nc.vector.tensor_mul(t1, f_a, cT)
                          t2 = tmp.tile([128, 3 * BC], f32, tag="t2")
                          nc.vector.tensor_mul(t2, i_a, g_a)
                          cT = cpool.tile([128, 3 * BC], f32, tag="c")
                          nc.vector.tensor_add(cT, t1, t2)
                          tc2 = tmp.tile([128, 3 * BC], f32, tag="tc2")
                          nc.scalar.activation(tc2, cT, AF.Tanh)
                          hT = hpool.tile([128, 3 * BC], f32, tag="h")
                          nc.vector.tensor_mul(hT, o_a, tc2)
                          nc.sync.dma_start(out=h2_d.ap()[:, :, t * BC:(t + 1) * BC], in_=hT)

            # ================= Phase C: logits + log_softmax =================
            with ExitStack() as pc:
              if 'c' in phases:
                  h2r = pc.enter_context(tc.tile_pool(name="h2r", bufs=3))
                  encr = pc.enter_context(tc.tile_pool(name="encr", bufs=3))
                  smp = pc.enter_context(tc.tile_pool(name="smp", bufs=4))
                  smc = pc.enter_context(tc.tile_pool(name="smc", bufs=6))
                  psc = pc.enter_context(tc.tile_pool(name="psC", bufs=4, space="PSUM"))

                  for ch in range(COLS // 128):
                      tc0 = ch * 8
                      h2t = h2r.tile([128, 3, 128], f32)
                      nc.sync.dma_start(out=h2t,
                                        in_=h2_d.ap()[:, :, ch * 128:(ch + 1) * 128])
                      enct = encr.tile([128, 6, 8, BC], f32)
                      for fc in range(6):
                          nc.sync.dma_start(out=enct[:, fc, :, :],
                                            in_=encT_d.ap()[fc, :, tc0:tc0 + 8, :])
                      ps = psc.tile([128, LABEL], f32)
                      for k in range(3):
                          nc.tensor.matmul(ps, h2t[:, k, :], outWhT_sb[:, k, :],
                                           start=(k == 0), stop=False)
                      for fc in range(6):
                          nc.tensor.matmul(ps, enct[:, fc, :, :],
                                           outWeT_sb[:, fc, :],
                                           start=False, stop=(fc == 5))
                      lg = smp.tile([128, LABEL], f32, tag="lg")
                      nc.vector.tensor_copy(lg, ps)
                      if ch == 0:
                          nc.vector.memset(lg[0:BC, APP_ID:APP_ID + 1], -1e10)
                      mx = smc.tile([128, 1], f32, tag="mx")
                      nc.vector.tensor_reduce(mx, lg, mybir.AxisListType.X, ALU.max)
                      xm = smp.tile([128, LABEL], f32, tag="xm")
                      nc.vector.tensor_scalar(xm, lg, mx, None, ALU.subtract)
                      et = smp.tile([128, LABEL], f32, tag="et")
                      ssum = smc.tile([128, 1], f32, tag="ssum")
                      nc.scalar.activation(et, xm, AF.Exp, accum_out=ssum)
                      lns = smc.tile([128, 1], f32, tag="lns")
                      nc.scalar.activation(lns, ssum, AF.Ln)
                      res = smp.tile([128, LABEL], f32, tag="res")
                      nc.vector.tensor_scalar(res, xm, lns, None, ALU.subtract)
                      nc.sync.dma_start(
                          out=out.ap().rearrange("b t l -> t b l")[tc0:tc0 + 8, :, :],
                          in_=res)

    nc.compile()
    return nc


def _host_prep(encoder_out, pos_embed_w, W_ih, W_hh, b_ih, b_hh,
               combine_W, combine_b, out_W, word_start, pos_ids):
    enc = np.ascontiguousarray(np.asarray(encoder_out, dtype=np.float32))
    ws = np.asarray(word_start)
    pid = np.asarray(pos_ids)
    tgrid = np.arange(T)[:, None]
    valid = ws >= 0
    s = np.clip(ws, 0, None)
    ln = np.maximum(tgrid - s, 1)
    recipv = (valid / ln).astype(np.float32)
    t0 = (tgrid // 128) * 128
    sreld = (s - t0).astype(np.float32)
    srelc = (s - t0 + 8).astype(np.float32)

    shared = dict(
        combWT=np.ascontiguousarray(
            np.asarray(combine_W, np.float32).T).reshape(7, 128, HID),
        wihT=np.ascontiguousarray(
            np.asarray(W_ih, np.float32).T).reshape(6, 128, 4 * H),
        whhT=np.ascontiguousarray(
            np.asarray(W_hh, np.float32).T).reshape(3, 128, 4 * H),
        outWhT=np.ascontiguousarray(
            np.asarray(out_W, np.float32)[:, :H].T).reshape(3, 128, LABEL),
        outWeT=np.ascontiguousarray(
            np.asarray(out_W, np.float32)[:, H:].T).reshape(6, 128, LABEL),
        posw=np.asarray(pos_embed_w, np.float32),
        combb=np.asarray(combine_b, np.float32).reshape(6, 128),
        biassum=(np.asarray(b_ih, np.float32)
                 + np.asarray(b_hh, np.float32)).reshape(12, 128),
        mlt=(np.arange(128)[:, None] < np.arange(128)[None, :]
             ).astype(np.float32),
        iota=np.arange(128, dtype=np.float32),
        id128=np.eye(128, dtype=np.float32),
    )
    in_maps = []
    for c in range(NCORES):
        bs = slice(c * BC, (c + 1) * BC)
        m = dict(shared)
        m["enc"] = np.ascontiguousarray(enc[bs])
        m["sreld"] = np.ascontiguousarray(sreld[:, bs].T)
        m["srelc"] = np.ascontiguousarray(srelc[:, bs].T)
        m["recipv"] = np.ascontiguousarray(recipv[:, bs].T)
        m["pidcol"] = np.ascontiguousarray(
            pid[:, bs].astype(np.float32).reshape(-1))
        in_maps.append(m)
    return in_maps


def _get_compiled():
    global _COMPILED
    if _COMPILED is None:
        import os
        reps = int(os.environ.get("BK_REPS", "1"))
        phases = os.environ.get("BK_PHASES", "abc")
        _COMPILED = _build(reps=reps, phases=phases)
    return _COMPILED


def kernel(**inputs):
    from concourse.bass_utils import run_bass_kernel_spmd
    nc = _get_compiled()
    in_maps = _host_prep(**inputs)
    res = run_bass_kernel_spmd(nc, in_maps, list(range(NCORES)))
    outs = [res.results[c]["out"] for c in range(NCORES)]
    full = np.concatenate(outs, axis=0)           # [B, T, LABEL]
    return full.reshape(B * T, LABEL).astype(np.float32)



# revision 2
# speedup vs baseline: 1.0886x; 1.0886x over previous
"""Trainium2 Bass kernel for nn_Decoder (ragged LSTM decoder), 8-core SPMD, v2.

Data-parallel over batch (BC=16 rows/core). Column layout everywhere is
t-major: col = t*16 + b_local. All matmul inputs bf16 (host-converted),
PSUM fp32, output fp32.

Per core, pipelined per tt (128 timesteps):
  A (parallel over t): ragged word-avg via banded matmuls against
    host-built coefficient tiles in (j,b)-packed layout; pos-embedding
    gathered on host; z = tanh(x @ combine_W.T + b); G_in = z @ W_ih.T
    + (b_ih + b_hh) -> DRAM (bf16), gates permuted [i,f,o,g].
    Also enc^T tiles (PE transpose) -> DRAM for phase C.
  B (sequential scan): per step, gates = G_in[t] (injected into PSUM via
    identity matmul) + h @ W_hh.T (36 128x128xbf16 matmuls, N=16);
    sigmoid/tanh on ACT, cell math on DVE; h history -> DRAM per block.
    A-work for tt+1 is interleaved between scan steps to fill engine
    idle time.
  C (parallel): logits = [h2, e_t] @ out_W.T in [(t,b), label] layout,
    t==0 appID fixup, log_softmax along free axis, DMA out fp32.
"""
import sys
sys.path.insert(0, "/opt/trn_rl_repo")

import numpy as np
import ml_dtypes

BF16 = ml_dtypes.bfloat16

B, T, H = 128, 512, 384
D_ENC, HID = 768, 768
POS_SIZE, POS_DIM, LABEL = 64, 128, 128
APP_ID = 3
NCORES = 8
BC = B // NCORES          # 16 batch rows per core
COLS = T * BC             # 8192 (t-major)
NG = T // 8               # 64 groups of 8 timesteps
NCH = COLS // 512         # 16 column chunks of 512
NTT = 4                   # 4 tt blocks of 128 timesteps

_COMPILED = None


def _build():
    import concourse.bass as bass
    import concourse.mybir as mybir
    import concourse.tile as tile
    from concourse import bacc
    from contextlib import ExitStack

    f32 = mybir.dt.float32
    bf16 = mybir.dt.bfloat16
    AF = mybir.ActivationFunctionType
    ALU = mybir.AluOpType

    nc = bacc.Bacc(None, target_bir_lowering=False, debug=False,
                   num_devices=NCORES)

    def param(name, shape, dt=bf16):
        return nc.declare_dram_parameter(name, list(shape), dt, isOutput=False)

    encJB = param("encJB", [NG, 8, BC, D_ENC])          # [g, j, b, d]
    cmat = param("cmat", [NG, 128, 256])                # [g, (j,b), lo|hi (t,b)]
    posT = param("posT", [128, COLS])
    combWT = param("combWT", [7, 128, HID])
    wihT = param("wihT", [6, 128, 4 * H])               # gate order [i,f,o,g]
    whhT = param("whhT", [3, 128, 4 * H])
    outWT = param("outWT", [9, 128, LABEL])
    id128 = param("id128", [128, 128])
    combb = param("combb", [6, 128], f32)
    biassum = param("biassum", [12, 128], f32)

    out = nc.declare_dram_parameter("out", [BC, T, LABEL], f32, isOutput=True)

    gin = [nc.dram_tensor(f"gin{tt}", [128, 8, 12, 16, BC], bf16)
           for tt in range(NTT)]
    h2_d = nc.dram_tensor("h2_d", [128, 3, T, BC], bf16)
    encT_d = nc.dram_tensor("encT_d", [6, 128, T, BC], bf16)

    with tile.TileContext(nc) as tc, ExitStack() as top:
        singles = top.enter_context(tc.tile_pool(name="singles", bufs=1))

        combWT_sb = singles.tile([128, 7, HID], bf16)
        wihT_sb = singles.tile([128, 6, 4 * H], bf16)
        whhT_sb = singles.tile([128, 3, 4 * H], bf16)
        outWT_sb = singles.tile([128, 9, LABEL], bf16)
        id_sb = singles.tile([128, 128], bf16)
        combb_sb = singles.tile([128, 6], f32)
        biassum_sb = singles.tile([128, 12], f32)
        h0_sb = singles.tile([128, 3 * BC], bf16)
        c0_sb = singles.tile([128, 3 * BC], f32)
        nc.sync.dma_start(out=combWT_sb, in_=combWT.ap().rearrange("k p m -> p k m"))
        nc.sync.dma_start(out=wihT_sb, in_=wihT.ap().rearrange("k p m -> p k m"))
        nc.sync.dma_start(out=whhT_sb, in_=whhT.ap().rearrange("k p m -> p k m"))
        nc.sync.dma_start(out=outWT_sb, in_=outWT.ap().rearrange("k p m -> p k m"))
        nc.sync.dma_start(out=id_sb, in_=id128.ap())
        nc.sync.dma_start(out=combb_sb, in_=combb.ap().rearrange("m p -> p m"))
        nc.sync.dma_start(out=biassum_sb, in_=biassum.ap().rearrange("m p -> p m"))
        nc.vector.memset(h0_sb, 0.0)
        nc.vector.memset(c0_sb, 0.0)

        with ExitStack() as ab:
            # ---- phase A pools ----
            p_pool = ab.enter_context(tc.tile_pool(name="pP", bufs=6))
            cm_pool = ab.enter_context(tc.tile_pool(name="pCM", bufs=6))
            xt_pool = ab.enter_context(tc.tile_pool(name="pXT", bufs=2))
            zt_pool = ab.enter_context(tc.tile_pool(name="pZT", bufs=2))
            go_pool = ab.enter_context(tc.tile_pool(name="pGO", bufs=4))
            es_pool = ab.enter_context(tc.tile_pool(name="pES", bufs=4))
            psA = ab.enter_context(tc.tile_pool(name="psA", bufs=2, space="PSUM"))
            # ---- phase B pools ----
            gch_pool = ab.enter_context(tc.tile_pool(name="pGCH", bufs=3))
            hb_pool = ab.enter_context(tc.tile_pool(name="pHB", bufs=2))
            c_pool = ab.enter_context(tc.tile_pool(name="pC", bufs=2))
            ga_pool = ab.enter_context(tc.tile_pool(name="pGA", bufs=2))
            tg_pool = ab.enter_context(tc.tile_pool(name="pTG", bufs=2))
            tmp_pool = ab.enter_context(tc.tile_pool(name="pTMP", bufs=4))
            psBg = ab.enter_context(tc.tile_pool(name="psBg", bufs=1, space="PSUM"))
            psBi = ab.enter_context(tc.tile_pool(name="psBi", bufs=1, space="PSUM"))

            # P tiles carried across chunks: keyed by group index
            p_tiles = {}

            def emit_chunk_units(cc):
                """Return a list of closures; running all emits phase A for
                column chunk cc (512 cols = 32 timesteps)."""
                units = []
                groups = [cc * 4 + i for i in range(4)]
                ctx = {}  # lazily allocated per-chunk tiles (at unit run time)

                def get_xT():
                    if 'xT' not in ctx:
                        ctx['xT'] = xt_pool.tile([128, 7, 512], bf16, tag="xT")
                    return ctx['xT']

                def get_zT():
                    if 'zT' not in ctx:
                        ctx['zT'] = zt_pool.tile([128, 6, 512], bf16, tag="zT")
                    return ctx['zT']

                def u_pdma(g):
                    def f():
                        pt = p_pool.tile([128, D_ENC], bf16, tag="P")
                        p_tiles[g] = pt
                        nc.sync.dma_start(
                            out=pt,
                            in_=encJB.ap()[g].rearrange("j b d -> (j b) d"))
                        cmt = cm_pool.tile([128, 256], bf16, tag="CM")
                        p_tiles[(g, 'c')] = cmt
                        nc.sync.dma_start(out=cmt, in_=cmat.ap()[g])
                    return f

                def u_pos():
                    def f():
                        nc.sync.dma_start(
                            out=get_xT()[:, 0, :],
                            in_=posT.ap()[:, cc * 512:(cc + 1) * 512])
                    return f

                def u_word(dc):
                    def f():
                        ps = psA.tile([128, 512], f32, tag="psA")
                        for i, g in enumerate(groups):
                            cmt = p_tiles[(g, 'c')]
                            sl = ps[:, i * 128:(i + 1) * 128]
                            if g == 0:
                                nc.tensor.matmul(
                                    sl, p_tiles[g][:, dc * 128:(dc + 1) * 128],
                                    cmt[:, 128:256], start=True, stop=True)
                            else:
                                nc.tensor.matmul(
                                    sl, p_tiles[g - 1][:, dc * 128:(dc + 1) * 128],
                                    cmt[:, 0:128], start=True, stop=False)
                                nc.tensor.matmul(
                                    sl, p_tiles[g][:, dc * 128:(dc + 1) * 128],
                                    cmt[:, 128:256], start=False, stop=True)
                        xT = get_xT()
                        if dc % 2 == 0:
                            nc.vector.tensor_copy(xT[:, 1 + dc, :], ps)
                        else:
                            nc.scalar.activation(xT[:, 1 + dc, :], ps, AF.Copy)
                    return f

                def u_trans(dc):
                    def f():
                        ps = psA.tile([128, 512], f32, tag="psA")
                        for i, g in enumerate(groups):
                            nc.tensor.matmul(
                                ps[:, i * 128:(i + 1) * 128],
                                p_tiles[g][:, dc * 128:(dc + 1) * 128],
                                id_sb, start=True, stop=True)
                        es = es_pool.tile([128, 32, BC], bf16, tag="eS")
                        if dc % 2 == 0:
                            nc.scalar.activation(es, ps, AF.Copy)
                        else:
                            nc.vector.tensor_copy(es, ps)
                        nc.sync.dma_start(
                            out=encT_d.ap()[dc, :, cc * 32:(cc + 1) * 32, :],
                            in_=es)
                    return f

                def u_z(m):
                    def f():
                        ps = psA.tile([128, 512], f32, tag="psA")
                        xT = get_xT()
                        for k in range(7):
                            nc.tensor.matmul(
                                ps, combWT_sb[:, k, m * 128:(m + 1) * 128],
                                xT[:, k, :], start=(k == 0), stop=(k == 6))
                        zT = get_zT()
                        nc.scalar.activation(zT[:, m, :], ps, AF.Tanh,
                                             bias=combb_sb[:, m:m + 1])
                        if cc == 0:
                            nc.vector.memset(zT[:, m, 0:BC], 0.0)
                    return f

                def u_g(j):
                    def f():
                        ps = psA.tile([128, 512], f32, tag="psA")
                        zT = get_zT()
                        for k in range(6):
                            nc.tensor.matmul(
                                ps, wihT_sb[:, k, j * 128:(j + 1) * 128],
                                zT[:, k, :], start=(k == 0), stop=(k == 5))
                        go = go_pool.tile([128, 2, 16, BC], bf16, tag="gO")
                        if j % 2 == 0:
                            nc.scalar.activation(go, ps, AF.Identity,
                                                 bias=biassum_sb[:, j:j + 1])
                        else:
                            nc.vector.tensor_scalar(go, ps,
                                                    biassum_sb[:, j:j + 1],
                                                    None, ALU.add)
                        tt, blk2 = divmod(cc, 4)
                        nc.sync.dma_start(
                            out=gin[tt].ap()[:, blk2 * 2:blk2 * 2 + 2, j, :, :],
                            in_=go)
                    return f

                for g in groups:
                    units.append(u_pdma(g))
                units.append(u_pos())
                for dc in range(6):
                    units.append(u_word(dc))
                    units.append(u_trans(dc))
                for m in range(6):
                    units.append(u_z(m))
                for j in range(12):
                    units.append(u_g(j))
                return units

            def emit_tt_units(tt):
                units = []
                for c4 in range(4):
                    units.extend(emit_chunk_units(tt * 4 + c4))
                return units

            # ---- prologue: phase A for tt=0, serial ----
            for u in emit_tt_units(0):
                u()

            # ---- phase B scan, interleaved with A(tt+1) ----
            h_prev = [h0_sb[:, k * BC:(k + 1) * BC] for k in range(3)]
            c_prev = c0_sb
            for tt in range(NTT):
                nxt = emit_tt_units(tt + 1) if tt + 1 < NTT else []
                emitted = 0
                for blk in range(8):
                    gch = gch_pool.tile([128, 12, 16, BC], bf16, tag="gch")
                    nc.sync.dma_start(out=gch, in_=gin[tt].ap()[:, blk])
                    hblk = hb_pool.tile([128, 16, 3 * BC], bf16, tag="hblk")
                    for s in range(16):
                        psg = psBg.tile([128, 3 * BC], f32, tag="psg")
                        psi = psBi.tile([128, 9 * BC], f32, tag="psi")
                        # g-gate bank: inject G_in then accumulate W_hh @ h
                        nc.tensor.matmul(psg, id_sb, gch[:, 9:12, s, :],
                                         start=True, stop=False,
                                         skip_group_check=True)
                        for jj in range(3):
                            j = 9 + jj
                            for k in range(3):
                                nc.tensor.matmul(
                                    psg[:, jj * BC:(jj + 1) * BC],
                                    whhT_sb[:, k, j * 128:(j + 1) * 128],
                                    h_prev[k],
                                    start=False,
                                    stop=(jj == 2 and k == 2),
                                    skip_group_check=True)
                        # i,f,o bank
                        nc.tensor.matmul(psi, id_sb, gch[:, 0:9, s, :],
                                         start=True, stop=False,
                                         skip_group_check=True)
                        for j in range(9):
                            for k in range(3):
                                nc.tensor.matmul(
                                    psi[:, j * BC:(j + 1) * BC],
                                    whhT_sb[:, k, j * 128:(j + 1) * 128],
                                    h_prev[k],
                                    start=False,
                                    stop=(j == 8 and k == 2),
                                    skip_group_check=True)
                        tanhg = tg_pool.tile([128, 3 * BC], bf16, tag="tanhg")
                        nc.scalar.activation(tanhg, psg, AF.Tanh)
                        gact = ga_pool.tile([128, 9 * BC], bf16, tag="gact")
                        nc.scalar.activation(gact, psi, AF.Sigmoid)
                        t1 = tmp_pool.tile([128, 3 * BC], f32, tag="t1")
                        nc.vector.tensor_mul(t1, gact[:, 3 * BC:6 * BC], c_prev)
                        t2 = tmp_pool.tile([128, 3 * BC], f32, tag="t2")
                        nc.vector.tensor_mul(t2, gact[:, 0:3 * BC], tanhg)
                        c_new = c_pool.tile([128, 3 * BC], f32, tag="c")
                        nc.vector.tensor_add(c_new, t1, t2)
                        tc2 = tmp_pool.tile([128, 3 * BC], bf16, tag="tc2")
                        nc.scalar.activation(tc2, c_new, AF.Tanh)
                        nc.vector.tensor_mul(hblk[:, :, s, :],
                                             gact[:, 6 * BC:9 * BC], tc2)
                        h_prev = [hblk[:, k, s, :] for k in range(3)]
                        c_prev = c_new
                        # interleave A(tt+1) units
                        step_i = blk * 16 + s + 1
                        target = step_i * len(nxt) // 128
                        while emitted < target:
                            nxt[emitted]()
                            emitted += 1
                    t0 = tt * 128 + blk * 16
                    nc.sync.dma_start(out=h2_d.ap()[:, :, t0:t0 + 16, :],
                                      in_=hblk)
                while emitted < len(nxt):
                    nxt[emitted]()
                    emitted += 1

        # ---- phase C: logits + log_softmax ----
        with ExitStack() as pc:
            h2r = pc.enter_context(tc.tile_pool(name="h2r", bufs=3))
            encr = pc.enter_context(tc.tile_pool(name="encr", bufs=3))
            smp = pc.enter_context(tc.tile_pool(name="smp", bufs=4))
            smc = pc.enter_context(tc.tile_pool(name="smc", bufs=6))
            psC = pc.enter_context(tc.tile_pool(name="psC", bufs=4, space="PSUM"))

            for pair in range(T // 16):
                h2t = h2r.tile([128, 16, 3 * BC], bf16, tag="h2t")
                nc.sync.dma_start(out=h2t,
                                  in_=h2_d.ap()[:, pair * 16:(pair + 1) * 16, :])
                enct = encr.tile([128, 6, 16, BC], bf16, tag="enct")
                nc.sync.dma_start(
                    out=enct,
                    in_=encT_d.ap()[:, :, pair * 16:(pair + 1) * 16, :]
                    .rearrange("dc p t b -> p dc t b"))
                for half in range(2):
                    ch = pair * 2 + half
                    toff = half * 8
                    ps = psC.tile([128, LABEL], f32, tag="psC")
                    for k in range(3):
                        nc.tensor.matmul(
                            ps, h2t[:, k, toff:toff + 8, :],
                            outWT_sb[:, k, :], start=(k == 0), stop=False)
                    for dc in range(6):
                        nc.tensor.matmul(
                            ps, enct[:, dc, toff:toff + 8, :],
                            outWT_sb[:, 3 + dc, :],
                            start=False, stop=(dc == 5))
                    lg = smp.tile([128, LABEL], f32, tag="lg")
                    nc.vector.tensor_copy(lg, ps)
                    if ch == 0:
                        nc.vector.memset(lg[0:BC, APP_ID:APP_ID + 1], -1e10)
                    mx = smc.tile([128, 1], f32, tag="mx")
                    nc.vector.tensor_reduce(mx, lg, mybir.AxisListType.X,
                                            ALU.max)
                    xm = smp.tile([128, LABEL], f32, tag="xm")
                    nc.vector.tensor_scalar(xm, lg, mx, None, ALU.subtract)
                    et = smp.tile([128, LABEL], bf16, tag="et")
                    ssum = smc.tile([128, 1], f32, tag="ssum")
                    nc.scalar.activation(et, xm, AF.Exp, accum_out=ssum)
                    lns = smc.tile([128, 1], f32, tag="lns")
                    nc.scalar.activation(lns, ssum, AF.Ln)
                    res = smp.tile([128, LABEL], f32, tag="res")
                    nc.vector.tensor_scalar(res, xm, lns, None, ALU.subtract)
                    nc.sync.dma_start(
                        out=out.ap().rearrange("b t l -> t b l")[
                            ch * 8:(ch + 1) * 8, :, :],
                        in_=res)

    nc.compile()
    return nc


def _host_prep(encoder_out, pos_embed_w, W_ih, W_hh, b_ih, b_hh,
               combine_W, combine_b, out_W, word_start, pos_ids):
    enc = np.asarray(encoder_out, dtype=np.float32)       # [B, T, D]
    ws = np.asarray(word_start)                           # [T, B]
    pid = np.asarray(pos_ids)                             # [T, B]
    pw = np.asarray(pos_embed_w, np.float32)

    tgrid = np.arange(T)[:, None]
    valid = ws >= 0
    s = np.clip(ws, 0, None)
    ln = np.maximum(tgrid - s, 1)
    recip = (valid / ln).astype(np.float32)               # [T, B]

    # gate permutation [i, f, g, o] -> [i, f, o, g]
    perm = np.r_[0:2 * H, 3 * H:4 * H, 2 * H:3 * H]

    shared = dict(
        combWT=np.ascontiguousarray(
            np.asarray(combine_W, np.float32).T).reshape(7, 128, HID
                                                         ).astype(BF16),
        wihT=np.ascontiguousarray(
            np.asarray(W_ih, np.float32)[perm].T).reshape(6, 128, 4 * H
                                                          ).astype(BF16),
        whhT=np.ascontiguousarray(
            np.asarray(W_hh, np.float32)[perm].T).reshape(3, 128, 4 * H
                                                          ).astype(BF16),
        outWT=np.concatenate([
            np.ascontiguousarray(
                np.asarray(out_W, np.float32)[:, :H].T).reshape(3, 128, LABEL),
            np.ascontiguousarray(
                np.asarray(out_W, np.float32)[:, H:].T).reshape(6, 128, LABEL),
        ], axis=0).astype(BF16),
        id128=np.eye(128, dtype=np.float32).astype(BF16),
        combb=np.asarray(combine_b, np.float32).reshape(6, 128),
        biassum=(np.asarray(b_ih, np.float32)
                 + np.asarray(b_hh, np.float32))[perm].reshape(12, 128),
    )

    # banded coefficient tiles, shared structure computed once per core below
    toff = np.arange(8)
    g_idx = np.arange(NG)
    tg = (g_idx[:, None] * 8 + toff[None, :])             # [NG, 8] global t
    jlo = (g_idx[:, None] * 8 - 8 + toff[None, :])        # [NG, 8] j of lo tile
    jhi = (g_idx[:, None] * 8 + toff[None, :])            # [NG, 8] j of hi tile
    eye_bb = np.eye(BC, dtype=np.float32)

    in_maps = []
    for c in range(NCORES):
        bs = slice(c * BC, (c + 1) * BC)
        m = dict(shared)
        m["encJB"] = np.ascontiguousarray(
            enc[bs].transpose(1, 0, 2).reshape(NG, 8, BC, D_ENC)).astype(BF16)
        m["posT"] = np.ascontiguousarray(
            pw[pid[:, bs]].transpose(2, 0, 1).reshape(POS_DIM, COLS)
        ).astype(BF16)
        s_c = s[:, bs]                                    # [T, BC]
        r_c = recip[:, bs]                                # [T, BC]
        # mask[g, j_off, t_off, b] = s[t] <= j < t  (t = tg, j = jlo/jhi)
        s_g = s_c.reshape(NG, 8, BC)                      # [g, t_off, b]
        r_g = r_c.reshape(NG, 8, BC)
        mlo = ((jlo[:, :, None, None] >= s_g[:, None, :, :])
               & (jlo[:, :, None, None] < tg[:, None, :, None]))
        mhi = ((jhi[:, :, None, None] >= s_g[:, None, :, :])
               & (jhi[:, :, None, None] < tg[:, None, :, None]))
        # cmat[g, (j_off,b), (t_off,b')] = mask * recip * (b==b')
        clo = (mlo * r_g[:, None, :, :])                  # [g, j_off, t_off, b]
        chi = (mhi * r_g[:, None, :, :])
        clo_t = np.einsum('gjtb,bc->gjbtc', clo, eye_bb)  # [g,j,b,t,b']
        chi_t = np.einsum('gjtb,bc->gjbtc', chi, eye_bb)
        cm = np.concatenate([clo_t.reshape(NG, 128, 128),
                             chi_t.reshape(NG, 128, 128)], axis=2)
        m["cmat"] = np.ascontiguousarray(cm).astype(BF16)
        in_maps.append(m)
    return in_maps


def _get_compiled():
    global _COMPILED
    if _COMPILED is None:
        _COMPILED = _build()
    return _COMPILED


def kernel(**inputs):
    from concourse.bass_utils import run_bass_kernel_spmd
    nc = _get_compiled()
    in_maps = _host_prep(**inputs)
    res = run_bass_kernel_spmd(nc, in_maps, list(range(NCORES)))
    outs = [res.results[c]["out"] for c in range(NCORES)]
    full = np.concatenate(outs, axis=0)           # [B, T, LABEL]
    return full.reshape(B * T, LABEL).astype(np.float32)
